# revision 18
# baseline (speedup 1.0000x reference)
"""MoE layer (dense all-experts SwiGLU + router-weighted sum) on 8 TRN2 cores.

Expert-parallel: core e holds expert e's weights (E=8). Every core sees the
full token stream x (shipped pre-transposed as xT [H, N]) and computes
  y_e = softmax(x @ W_router)[:, e] * ((silu(x@Wg_e) * (x@Wu_e)) @ Wd_e)
The host sums the 8 per-expert outputs.

All matmul operands are bf16 (inputs quantized on host, ~1e-3 rel err vs
the 2e-2 gate), PSUM accumulation fp32. bf16 halves SBUF so ALL weights
(Wg, Wu, Wd) are resident: no per-block weight streaming at all. The only
steady-state DMA is xt in (1 MB/block, sync HWDGE ring, 3 blocks deep) and
y out (2 MB/block, riding the otherwise-idle SWDGE ring; the last block
rides sync so the tail drains fast).

Per-core program, per 512-token block:
  router: logits^T [8,512] via PE (Wr stationary, xT moving), Exp on ACT,
          then per 128-token subtile a transpose-matmul with rhs=[ones|e_sel]
          gives [denom | numer] in PSUM -> w = numer * 1/denom on DVE.
  stage1: G/U [128i, 512tok] = Wg/Wu_chunk^T @ xT_chunk (8 K-chunks in PSUM),
          hT[i] = silu(G)*U -> SBUF bf16 (resident for the block, [I, tok]).
  stage2: Y[m] [128tok, 512h] accumulates over 16 i-chunks with hT as
          stationary and resident Wd slices as moving; evict = DVE multiply
          by the router weight, y DMA issued per tile.

Block 0 runs stage 1 k-OUTER in groups of 4 i-chunks (borrowing the psy
PSUM banks) so compute starts as soon as the first 512KB Wg k-chunk lands
(~5us) instead of waiting for the full Wg (~16us). Weight DMAs are issued
k-chunk-granular in consumption order: wr, xt(0), wg, xt(1), wu, wd, xt(2).

Scheduling notes:
  - tile-pool slot allocation order must match consumption order or the
    schedule deadlocks; all 8 xt chunks of 3 blocks are live at once.
  - next block's router runs between the two h-sweeps (PE filler).
"""
import numpy as np

import concourse.bass as bass
import concourse.mybir as mybir
import concourse.tile as tile
from concourse import bacc
from concourse.bass_utils import run_bass_kernel_spmd

P = 128
H, I, E = 1024, 2048, 8
N = 8192  # tokens = 4 * 2048
HK = H // P   # 8 contraction chunks over H
IK = I // P   # 16 chunks over I
TB = 512      # token block
NB = N // TB  # 16 blocks
NM = TB // P  # 4 token subtiles per block
NH = H // 512  # 2 output column halves
PF = 3        # xt prefetch depth (blocks)

F32 = mybir.dt.float32
DT16 = mybir.dt.float16
AF = mybir.ActivationFunctionType

# set by a driver (test.py) to profile; harness path keeps defaults
TRACE = False
LAST_EXEC_NS = None

_CACHE = {}


def _build():
    nc = bacc.Bacc("TRN2", target_bir_lowering=False, debug=False)

    xt_d = nc.dram_tensor("xt", [H, N], DT16, kind="ExternalInput").ap()
    wg_d = nc.dram_tensor("wg", [H, I], DT16, kind="ExternalInput").ap()
    wu_d = nc.dram_tensor("wu", [H, I], DT16, kind="ExternalInput").ap()
    wd_d = nc.dram_tensor("wd", [I, H], DT16, kind="ExternalInput").ap()
    wr_d = nc.dram_tensor("wr", [P, HK * E], DT16, kind="ExternalInput").ap()
    sel_d = nc.dram_tensor("sel", [P, 2 * E], DT16, kind="ExternalInput").ap()
    y_d = nc.dram_tensor("y", [N, H], DT16, kind="ExternalOutput").ap()

    with tile.TileContext(nc) as tc:
        with (
            tc.tile_pool(name="const", bufs=1) as const,
            tc.tile_pool(name="xtp", bufs=PF * HK) as xtp,
            tc.tile_pool(name="htp", bufs=1) as htp,
            tc.tile_pool(name="evp", bufs=8) as evp,
            tc.tile_pool(name="rtp", bufs=2) as rtp,
            tc.tile_pool(name="wp", bufs=2) as wp,
            tc.tile_pool(name="psgu", bufs=2, space="PSUM") as psgu,
            tc.tile_pool(name="psy", bufs=5, space="PSUM") as psy,
            tc.tile_pool(name="psr", bufs=1, space="PSUM") as psr,
        ):
            # resident weights: [128, HK*I] with chunk k at cols [k*I, (k+1)*I)
            wg_sb = const.tile([P, HK * I], DT16)
            wu_sb = const.tile([P, HK * I], DT16)
            # wd resident: [128, IK*H] with i-chunk at cols [i*H, (i+1)*H)
            wd_sb = const.tile([P, IK * H], DT16)
            wr_sb = const.tile([P, HK * E], DT16)
            sel_sb = const.tile([P, 2 * E], DT16)

            def load_xt(b, eng=None):
                eng = eng or nc.sync
                tok = slice(b * TB, (b + 1) * TB)
                chunks = []
                for k in range(HK):
                    ch = xtp.tile([P, TB], DT16, tag="xt", name=f"xt{b}_{k}")
                    eng.dma_start(
                        out=ch[:], in_=xt_d[k * P:(k + 1) * P, tok]
                    )
                    chunks.append(ch)
                return chunks

            # ---- PE warmup: the first input DMA cannot complete before
            # ~7-10us (DGE ring wake), and HAM holds the PE at 1.2GHz for
            # its first ~3.4us of activity. Burn both on dummy matmuls over
            # a memset tile so the real stream starts warm at 2.4GHz.
            wst = const.tile([P, TB], DT16)
            nc.vector.memset(wst[:], 0.0)
            wups = psr.tile([P, TB], F32, tag="rt", name="wups")
            for _ in range(36):
                nc.tensor.matmul(
                    wups[:], (wst[:, 0:P]), (wst[:]), start=True, stop=True
                )

            # ---- prologue DMAs in consumption order. The HWDGE rings take
            # ~7us to wake at kernel start while SWDGE delivers in ~3us, so
            # the block-0-critical wr/xt0/sel ride SWDGE; the bulk weight
            # stream (wg halves, xt1, wu halves, wd) rides the sync ring.
            IH = I // 2
            nc.gpsimd.dma_start(out=wr_sb[:], in_=wr_d[:])
            xt_next = load_xt(0, eng=nc.gpsimd)
            nc.gpsimd.dma_start(out=sel_sb[:], in_=sel_d[:])
            # wg streamed as 256KB half-chunks, half-A (i 0-7) of every k
            # first: block-0's k-outer groups consume them as they land.
            for half in range(2):
                for k in range(HK):
                    nc.sync.dma_start(
                        out=wg_sb[:, k * I + half * IH: k * I + (half + 1) * IH],
                        in_=wg_d[k * P:(k + 1) * P, half * IH:(half + 1) * IH],
                    )
            xt_pre1 = load_xt(1)
            for half in range(2):
                for k in range(HK):
                    nc.sync.dma_start(
                        out=wu_sb[:, k * I + half * IH: k * I + (half + 1) * IH],
                        in_=wu_d[k * P:(k + 1) * P, half * IH:(half + 1) * IH],
                    )
            for j in range(IK // 2):  # wd as 512KB 2-chunk batches
                rows = slice(2 * j * P, (2 * j + 2) * P)
                nc.sync.dma_start(
                    out=wd_sb[:, 2 * j * H:(2 * j + 2) * H].rearrange(
                        "p (j c) -> p j c", j=2),
                    in_=wd_d[rows, :].rearrange("(j p) c -> p j c", p=P),
                )

            def router(xt_ch):
                # w[tok] = softmax(logits)[:, e] for one block. Only the 8
                # lt matmuls touch the PE: exp'd logits move to token-
                # partition layout via DVE 32x32 block transposes, and
                # numer/denom come from DVE free-dim accumulations against
                # selbc ([ones | one-hot(e)] replicated per partition).
                lt = psr.tile([E, TB], F32, tag="rt", name="lt")
                for k in range(HK):
                    nc.tensor.matmul(
                        lt[:],
                        (wr_sb[:, k * E:(k + 1) * E]),
                        (xt_ch[k][:]),
                        start=(k == 0),
                        stop=(k == HK - 1),
                    )
                # zero all 32 rows first (partition base must be 32-
                # aligned, and the transposes must not read uninitialized
                # SBUF), then exp overwrites rows 0-7.
                exp_sb = rtp.tile([32, TB], DT16, tag="exp", name="exp_sb")
                nc.vector.memset(exp_sb[:], 0.0)
                nc.scalar.activation(exp_sb[0:E, :], lt[:], AF.Exp)
                wv = wp.tile([P, NM], F32, tag="wv", name="wv")
                for m in range(NM):
                    et = rtp.tile([P, 32], DT16, tag="et", name="et", bufs=4)
                    for j in range(4):
                        nc.vector.transpose(
                            out=et[j * 32:(j + 1) * 32, 0:32],
                            in_=exp_sb[0:32, m * P + j * 32:
                                       m * P + (j + 1) * 32],
                        )
                    junk = wp.tile([P, E], F32, tag="junk", name="junk")
                    den = wp.tile([P, 1], F32, tag="den", name="den")
                    nc.vector.scalar_tensor_tensor(
                        out=junk[:], in0=et[:, 0:E], scalar=1.0,
                        in1=sel_sb[:, 0:E], op0=mybir.AluOpType.mult,
                        op1=mybir.AluOpType.mult, accum_out=den[:],
                    )
                    num = wp.tile([P, 1], F32, tag="num", name="num")
                    nc.vector.scalar_tensor_tensor(
                        out=junk[:], in0=et[:, 0:E], scalar=1.0,
                        in1=sel_sb[:, E:2 * E], op0=mybir.AluOpType.mult,
                        op1=mybir.AluOpType.mult, accum_out=num[:],
                    )
                    rec = wp.tile([P, 1], F32, tag="rec", name="rec")
                    nc.vector.reciprocal(rec[:], den[:])
                    nc.vector.tensor_tensor(
                        out=wv[:, m:m + 1], in0=num[:], in1=rec[:],
                        op=mybir.AluOpType.mult,
                    )
                return wv

            xtq = [xt_next, xt_pre1]  # blocks b, b+1 (already issued)
            w_next = router(xtq[0])
            for b in range(NB):
                xt_ch = xtq.pop(0)
                w_tiles = w_next
                if b + 2 < NB:
                    xtq.append(load_xt(b + 2))

                # ---- stage 1: hT[i] = silu(G)*U, [I-chunk, tok] layout
                ht_sb = htp.tile([P, IK * TB], DT16, tag="ht")

                def g_step(i):
                    g_ps = psgu.tile([P, TB], F32, tag="gu", name="g_ps")
                    for k in range(HK):
                        nc.tensor.matmul(
                            g_ps[:],
                            (wg_sb[:, k * I + i * P: k * I + (i + 1) * P]),
                            (xt_ch[k][:]),
                            start=(k == 0),
                            stop=(k == HK - 1),
                        )
                    nc.scalar.activation(
                        ht_sb[:, i * TB:(i + 1) * TB], g_ps[:], AF.Silu
                    )

                def u_step(i):
                    u_ps = psgu.tile([P, TB], F32, tag="gu", name="u_ps")
                    for k in range(HK):
                        nc.tensor.matmul(
                            u_ps[:],
                            (wu_sb[:, k * I + i * P: k * I + (i + 1) * P]),
                            (xt_ch[k][:]),
                            start=(k == 0),
                            stop=(k == HK - 1),
                        )
                    hsl = ht_sb[:, i * TB:(i + 1) * TB]
                    nc.vector.tensor_tensor(
                        out=hsl, in0=hsl, in1=u_ps[:], op=mybir.AluOpType.mult
                    )

                if b == 0:
                    # k-outer in groups of 4 i-chunks (borrowing psy banks):
                    # each 512KB wg k-chunk unlocks 4 matmuls as it lands.
                    for w_sb, is_g in ((wg_sb, True), (wu_sb, False)):
                        for grp in range(IK // 4):
                            ps4 = [
                                psy.tile([P, TB], F32, tag="y",
                                         name=f"b0_{'g' if is_g else 'u'}{grp}_{j}")
                                for j in range(4)
                            ]
                            for k in range(HK):
                                for j in range(4):
                                    i = grp * 4 + j
                                    nc.tensor.matmul(
                                        ps4[j][:],
                                        (w_sb[:, k * I + i * P:
                                              k * I + (i + 1) * P]),
                                        (xt_ch[k][:]),
                                        start=(k == 0),
                                        stop=(k == HK - 1),
                                    )
                            for j in range(4):
                                i = grp * 4 + j
                                hsl = ht_sb[:, i * TB:(i + 1) * TB]
                                if is_g:
                                    nc.scalar.activation(hsl, ps4[j][:], AF.Silu)
                                else:
                                    nc.vector.tensor_tensor(
                                        out=hsl, in0=hsl, in1=ps4[j][:],
                                        op=mybir.AluOpType.mult,
                                    )
                else:
                    for i in range(IK):
                        g_step(i)
                        u_step(i)

                # ---- stage 2: Y[m] [128tok, 512h] = hT^T @ Wd (resident),
                # scaled by w on evict; y DMA per tile on the idle SWDGE ring.
                if b + 1 < NB:
                    for h in range(NH):
                        if h == 1:
                            # next block's router between the h-sweeps
                            w_next = router(xtq[0])
                        y_ps = [
                            psy.tile([P, 512], F32, tag="y", name=f"y_ps{m}")
                            for m in range(NM)
                        ]
                        for i in range(IK):
                            rhs = wd_sb[:, i * H + h * 512:
                                        i * H + (h + 1) * 512]
                            for m in range(NM):
                                nc.tensor.matmul(
                                    y_ps[m][:],
                                    (ht_sb[:, i * TB + m * P:
                                           i * TB + (m + 1) * P]),
                                    (rhs),
                                    start=(i == 0),
                                    stop=(i == IK - 1),
                                )
                        for m in range(NM):
                            y_sb = evp.tile([P, 512], DT16, tag="ev",
                                            name=f"yev{h}_{m}")
                            nc.vector.tensor_scalar_mul(
                                y_sb[:], y_ps[m][:], w_tiles[:, m:m + 1]
                            )
                            nc.gpsimd.dma_start(
                                out=y_d[b * TB + m * P: b * TB + (m + 1) * P,
                                        h * 512:(h + 1) * 512],
                                in_=y_sb[:],
                            )
                else:
                    # last block: m-outer so earlier m's evict+DMA (sync
                    # ring, now idle) overlap later m's matmuls -> short tail
                    for m in range(NM):
                        y2 = [
                            psy.tile([P, 512], F32, tag="y", name=f"yl{m}_{h}")
                            for h in range(NH)
                        ]
                        for i in range(IK):
                            st = ht_sb[:, i * TB + m * P: i * TB + (m + 1) * P]
                            for h in range(NH):
                                nc.tensor.matmul(
                                    y2[h][:],
                                    (st),
                                    (wd_sb[:, i * H + h * 512:
                                           i * H + (h + 1) * 512]),
                                    start=(i == 0),
                                    stop=(i == IK - 1),
                                )
                        for h in range(NH):
                            y_sb = evp.tile([P, 512], DT16, tag="ev",
                                            name=f"yevl{m}_{h}")
                            for q in range(2):
                                qs = slice(q * 256, (q + 1) * 256)
                                nc.vector.tensor_scalar_mul(
                                    y_sb[:, qs], y2[h][:, qs],
                                    w_tiles[:, m:m + 1]
                                )
                                nc.sync.dma_start(
                                    out=y_d[b * TB + m * P:
                                            b * TB + (m + 1) * P,
                                            h * 512 + q * 256:
                                            h * 512 + (q + 1) * 256],
                                    in_=y_sb[:, qs],
                                )

    nc.compile()
    return nc


def kernel(x, W_router, W_gate, W_up, W_down):
    global LAST_EXEC_NS
    if "nc" not in _CACHE:
        _CACHE["nc"] = _build()
    nc = _CACHE["nc"]

    bf16 = np.float16
    x_bf = np.asarray(x, dtype=np.float32).reshape(N, H).astype(bf16)
    xt = np.ascontiguousarray(x_bf.T)
    # repack router weights into the SBUF layout [128, HK*E]: row p holds
    # chunk k's rows (k*128+p) side by side -> plain contiguous DMA on device
    wr = np.ascontiguousarray(
        np.asarray(W_router, dtype=np.float32)
        .reshape(HK, P, E).transpose(1, 0, 2).reshape(P, HK * E)
    ).astype(bf16)
    eye = np.eye(E, dtype=np.float32)
    in_maps = []
    for e in range(E):
        # [ones | one-hot(e)] replicated across the 128 partitions
        sel = np.tile(
            np.concatenate([np.ones(E, dtype=np.float32), eye[e]]), (P, 1)
        )
        in_maps.append({
            "xt": xt,
            "wg": np.ascontiguousarray(W_gate[e]).astype(bf16),
            "wu": np.ascontiguousarray(W_up[e]).astype(bf16),
            "wd": np.ascontiguousarray(W_down[e]).astype(bf16),
            "wr": wr,
            "sel": np.ascontiguousarray(sel).astype(bf16),
        })

    res = run_bass_kernel_spmd(nc, in_maps, list(range(E)), trace=TRACE)
    LAST_EXEC_NS = res.exec_time_ns

    acc = np.zeros((N, H), dtype=np.float64)
    for r in res.results:
        acc += r["y"]
    return acc.astype(np.float32).reshape(x.shape[0], x.shape[1], H)


# revision 19
# speedup vs baseline: 1.0011x; 1.0011x over previous
"""MoE layer (dense all-experts SwiGLU + router-weighted sum) on 8 TRN2 cores.

Expert-parallel: core e holds expert e's weights (E=8). Every core sees the
full token stream x (shipped pre-transposed as xT [H, N]) and computes
  y_e = softmax(x @ W_router)[:, e] * ((silu(x@Wg_e) * (x@Wu_e)) @ Wd_e)
The host sums the 8 per-expert outputs.

All matmul operands are bf16 (inputs quantized on host, ~1e-3 rel err vs
the 2e-2 gate), PSUM accumulation fp32. bf16 halves SBUF so ALL weights
(Wg, Wu, Wd) are resident: no per-block weight streaming at all. The only
steady-state DMA is xt in (1 MB/block, sync HWDGE ring, 3 blocks deep) and
y out (2 MB/block, riding the otherwise-idle SWDGE ring; the last block
rides sync so the tail drains fast).

Per-core program, per 512-token block:
  router: logits^T [8,512] via PE (Wr stationary, xT moving), Exp on ACT,
          then per 128-token subtile a transpose-matmul with rhs=[ones|e_sel]
          gives [denom | numer] in PSUM -> w = numer * 1/denom on DVE.
  stage1: G/U [128i, 512tok] = Wg/Wu_chunk^T @ xT_chunk (8 K-chunks in PSUM),
          hT[i] = silu(G)*U -> SBUF bf16 (resident for the block, [I, tok]).
  stage2: Y[m] [128tok, 512h] accumulates over 16 i-chunks with hT as
          stationary and resident Wd slices as moving; evict = DVE multiply
          by the router weight, y DMA issued per tile.

Block 0 runs stage 1 k-OUTER in groups of 4 i-chunks (borrowing the psy
PSUM banks) so compute starts as soon as the first 512KB Wg k-chunk lands
(~5us) instead of waiting for the full Wg (~16us). Weight DMAs are issued
k-chunk-granular in consumption order: wr, xt(0), wg, xt(1), wu, wd, xt(2).

Scheduling notes:
  - tile-pool slot allocation order must match consumption order or the
    schedule deadlocks; all 8 xt chunks of 3 blocks are live at once.
  - next block's router runs between the two h-sweeps (PE filler).
"""
import numpy as np

import concourse.bass as bass
import concourse.mybir as mybir
import concourse.tile as tile
from concourse import bacc
from concourse.bass_utils import run_bass_kernel_spmd

P = 128
H, I, E = 1024, 2048, 8
N = 8192  # tokens = 4 * 2048
HK = H // P   # 8 contraction chunks over H
IK = I // P   # 16 chunks over I
TB = 512      # token block
NB = N // TB  # 16 blocks
NM = TB // P  # 4 token subtiles per block
NH = H // 512  # 2 output column halves
PF = 3        # xt prefetch depth (blocks)

F32 = mybir.dt.float32
DT16 = mybir.dt.float16
AF = mybir.ActivationFunctionType

# set by a driver (test.py) to profile; harness path keeps defaults
TRACE = False
LAST_EXEC_NS = None

_CACHE = {}


def _build():
    nc = bacc.Bacc("TRN2", target_bir_lowering=False, debug=False)

    xt_d = nc.dram_tensor("xt", [H, N], DT16, kind="ExternalInput").ap()
    wg_d = nc.dram_tensor("wg", [H, I], DT16, kind="ExternalInput").ap()
    wu_d = nc.dram_tensor("wu", [H, I], DT16, kind="ExternalInput").ap()
    wd_d = nc.dram_tensor("wd", [I, H], DT16, kind="ExternalInput").ap()
    wr_d = nc.dram_tensor("wr", [P, HK * E], DT16, kind="ExternalInput").ap()
    sel_d = nc.dram_tensor("sel", [P, 2 * E], DT16, kind="ExternalInput").ap()
    y_d = nc.dram_tensor("y", [N, H], DT16, kind="ExternalOutput").ap()

    with tile.TileContext(nc) as tc:
        with (
            tc.tile_pool(name="const", bufs=1) as const,
            tc.tile_pool(name="xtp", bufs=PF * HK) as xtp,
            tc.tile_pool(name="htp", bufs=1) as htp,
            tc.tile_pool(name="evp", bufs=8) as evp,
            tc.tile_pool(name="rtp", bufs=2) as rtp,
            tc.tile_pool(name="wp", bufs=2) as wp,
            tc.tile_pool(name="psgu", bufs=2, space="PSUM") as psgu,
            tc.tile_pool(name="psy", bufs=5, space="PSUM") as psy,
            tc.tile_pool(name="psr", bufs=1, space="PSUM") as psr,
        ):
            # resident weights: [128, HK*I] with chunk k at cols [k*I, (k+1)*I)
            wg_sb = const.tile([P, HK * I], DT16)
            wu_sb = const.tile([P, HK * I], DT16)
            # wd resident: [128, IK*H] with i-chunk at cols [i*H, (i+1)*H)
            wd_sb = const.tile([P, IK * H], DT16)
            wr_sb = const.tile([P, HK * E], DT16)
            sel_sb = const.tile([P, 2 * E], DT16)

            def load_xt(b, eng=None):
                eng = eng or nc.sync
                tok = slice(b * TB, (b + 1) * TB)
                chunks = []
                for k in range(HK):
                    ch = xtp.tile([P, TB], DT16, tag="xt", name=f"xt{b}_{k}")
                    eng.dma_start(
                        out=ch[:], in_=xt_d[k * P:(k + 1) * P, tok]
                    )
                    chunks.append(ch)
                return chunks

            # ---- PE warmup: the first input DMA cannot complete before
            # ~7-10us (DGE ring wake), and HAM holds the PE at 1.2GHz for
            # its first ~3.4us of activity. Burn both on dummy matmuls over
            # a memset tile so the real stream starts warm at 2.4GHz.
            wst = const.tile([P, TB], DT16)
            nc.vector.memset(wst[:], 0.0)
            wups = psr.tile([P, TB], F32, tag="rt", name="wups")
            for _ in range(24):
                nc.tensor.matmul(
                    wups[:], (wst[:, 0:P]), (wst[:]), start=True, stop=True
                )

            # ---- prologue DMAs in consumption order. The HWDGE rings take
            # ~7us to wake at kernel start while SWDGE delivers in ~3us, so
            # the block-0-critical wr/xt0/sel ride SWDGE; the bulk weight
            # stream (wg halves, xt1, wu halves, wd) rides the sync ring.
            IH = I // 2
            nc.gpsimd.dma_start(out=wr_sb[:], in_=wr_d[:])
            xt_next = load_xt(0, eng=nc.gpsimd)
            nc.gpsimd.dma_start(out=sel_sb[:], in_=sel_d[:])
            # wg streamed as 256KB half-chunks, half-A (i 0-7) of every k
            # first: block-0's k-outer groups consume them as they land.
            for half in range(2):
                for k in range(HK):
                    nc.sync.dma_start(
                        out=wg_sb[:, k * I + half * IH: k * I + (half + 1) * IH],
                        in_=wg_d[k * P:(k + 1) * P, half * IH:(half + 1) * IH],
                    )
            xt_pre1 = load_xt(1)
            for half in range(2):
                for k in range(HK):
                    nc.sync.dma_start(
                        out=wu_sb[:, k * I + half * IH: k * I + (half + 1) * IH],
                        in_=wu_d[k * P:(k + 1) * P, half * IH:(half + 1) * IH],
                    )
            for j in range(IK // 2):  # wd as 512KB 2-chunk batches
                rows = slice(2 * j * P, (2 * j + 2) * P)
                nc.sync.dma_start(
                    out=wd_sb[:, 2 * j * H:(2 * j + 2) * H].rearrange(
                        "p (j c) -> p j c", j=2),
                    in_=wd_d[rows, :].rearrange("(j p) c -> p j c", p=P),
                )

            def router(xt_ch):
                # w[tok] = softmax(logits)[:, e] for one block. Only the 8
                # lt matmuls touch the PE: exp'd logits move to token-
                # partition layout via DVE 32x32 block transposes, and
                # numer/denom come from DVE free-dim accumulations against
                # selbc ([ones | one-hot(e)] replicated per partition).
                lt = psr.tile([E, TB], F32, tag="rt", name="lt")
                for k in range(HK):
                    nc.tensor.matmul(
                        lt[:],
                        (wr_sb[:, k * E:(k + 1) * E]),
                        (xt_ch[k][:]),
                        start=(k == 0),
                        stop=(k == HK - 1),
                    )
                # zero all 32 rows first (partition base must be 32-
                # aligned, and the transposes must not read uninitialized
                # SBUF), then exp overwrites rows 0-7.
                exp_sb = rtp.tile([32, TB], DT16, tag="exp", name="exp_sb")
                nc.vector.memset(exp_sb[:], 0.0)
                nc.scalar.activation(exp_sb[0:E, :], lt[:], AF.Exp)
                wv = wp.tile([P, NM], F32, tag="wv", name="wv")
                for m in range(NM):
                    et = rtp.tile([P, 32], DT16, tag="et", name="et", bufs=4)
                    for j in range(4):
                        nc.vector.transpose(
                            out=et[j * 32:(j + 1) * 32, 0:32],
                            in_=exp_sb[0:32, m * P + j * 32:
                                       m * P + (j + 1) * 32],
                        )
                    junk = wp.tile([P, E], F32, tag="junk", name="junk")
                    den = wp.tile([P, 1], F32, tag="den", name="den")
                    nc.vector.scalar_tensor_tensor(
                        out=junk[:], in0=et[:, 0:E], scalar=1.0,
                        in1=sel_sb[:, 0:E], op0=mybir.AluOpType.mult,
                        op1=mybir.AluOpType.mult, accum_out=den[:],
                    )
                    num = wp.tile([P, 1], F32, tag="num", name="num")
                    nc.vector.scalar_tensor_tensor(
                        out=junk[:], in0=et[:, 0:E], scalar=1.0,
                        in1=sel_sb[:, E:2 * E], op0=mybir.AluOpType.mult,
                        op1=mybir.AluOpType.mult, accum_out=num[:],
                    )
                    rec = wp.tile([P, 1], F32, tag="rec", name="rec")
                    nc.vector.reciprocal(rec[:], den[:])
                    nc.vector.tensor_tensor(
                        out=wv[:, m:m + 1], in0=num[:], in1=rec[:],
                        op=mybir.AluOpType.mult,
                    )
                return wv

            xtq = [xt_next, xt_pre1]  # blocks b, b+1 (already issued)
            w_next = router(xtq[0])
            for b in range(NB):
                xt_ch = xtq.pop(0)
                w_tiles = w_next
                if b + 2 < NB:
                    xtq.append(load_xt(b + 2))

                # ---- stage 1: hT[i] = silu(G)*U, [I-chunk, tok] layout
                ht_sb = htp.tile([P, IK * TB], DT16, tag="ht")

                def g_step(i):
                    g_ps = psgu.tile([P, TB], F32, tag="gu", name="g_ps")
                    for k in range(HK):
                        nc.tensor.matmul(
                            g_ps[:],
                            (wg_sb[:, k * I + i * P: k * I + (i + 1) * P]),
                            (xt_ch[k][:]),
                            start=(k == 0),
                            stop=(k == HK - 1),
                        )
                    nc.scalar.activation(
                        ht_sb[:, i * TB:(i + 1) * TB], g_ps[:], AF.Silu
                    )

                def u_step(i):
                    u_ps = psgu.tile([P, TB], F32, tag="gu", name="u_ps")
                    for k in range(HK):
                        nc.tensor.matmul(
                            u_ps[:],
                            (wu_sb[:, k * I + i * P: k * I + (i + 1) * P]),
                            (xt_ch[k][:]),
                            start=(k == 0),
                            stop=(k == HK - 1),
                        )
                    hsl = ht_sb[:, i * TB:(i + 1) * TB]
                    nc.vector.tensor_tensor(
                        out=hsl, in0=hsl, in1=u_ps[:], op=mybir.AluOpType.mult
                    )

                if b == 0:
                    # k-outer in groups of 4 i-chunks (borrowing psy banks):
                    # each 512KB wg k-chunk unlocks 4 matmuls as it lands.
                    for w_sb, is_g in ((wg_sb, True), (wu_sb, False)):
                        for grp in range(IK // 4):
                            ps4 = [
                                psy.tile([P, TB], F32, tag="y",
                                         name=f"b0_{'g' if is_g else 'u'}{grp}_{j}")
                                for j in range(4)
                            ]
                            for k in range(HK):
                                for j in range(4):
                                    i = grp * 4 + j
                                    nc.tensor.matmul(
                                        ps4[j][:],
                                        (w_sb[:, k * I + i * P:
                                              k * I + (i + 1) * P]),
                                        (xt_ch[k][:]),
                                        start=(k == 0),
                                        stop=(k == HK - 1),
                                    )
                            for j in range(4):
                                i = grp * 4 + j
                                hsl = ht_sb[:, i * TB:(i + 1) * TB]
                                if is_g:
                                    nc.scalar.activation(hsl, ps4[j][:], AF.Silu)
                                else:
                                    nc.vector.tensor_tensor(
                                        out=hsl, in0=hsl, in1=ps4[j][:],
                                        op=mybir.AluOpType.mult,
                                    )
                else:
                    for i in range(IK):
                        g_step(i)
                        u_step(i)

                # ---- stage 2: Y[m] [128tok, 512h] = hT^T @ Wd (resident),
                # scaled by w on evict; y DMA per tile on the idle SWDGE ring.
                if b + 1 < NB:
                    for h in range(NH):
                        if h == 1:
                            # next block's router between the h-sweeps
                            w_next = router(xtq[0])
                        y_ps = [
                            psy.tile([P, 512], F32, tag="y", name=f"y_ps{m}")
                            for m in range(NM)
                        ]
                        for i in range(IK):
                            rhs = wd_sb[:, i * H + h * 512:
                                        i * H + (h + 1) * 512]
                            for m in range(NM):
                                nc.tensor.matmul(
                                    y_ps[m][:],
                                    (ht_sb[:, i * TB + m * P:
                                           i * TB + (m + 1) * P]),
                                    (rhs),
                                    start=(i == 0),
                                    stop=(i == IK - 1),
                                )
                        for m in range(NM):
                            y_sb = evp.tile([P, 512], DT16, tag="ev",
                                            name=f"yev{h}_{m}")
                            nc.vector.tensor_scalar_mul(
                                y_sb[:], y_ps[m][:], w_tiles[:, m:m + 1]
                            )
                            nc.gpsimd.dma_start(
                                out=y_d[b * TB + m * P: b * TB + (m + 1) * P,
                                        h * 512:(h + 1) * 512],
                                in_=y_sb[:],
                            )
                else:
                    # last block: m-outer so earlier m's evict+DMA (sync
                    # ring, now idle) overlap later m's matmuls -> short tail
                    for m in range(NM):
                        y2 = [
                            psy.tile([P, 512], F32, tag="y", name=f"yl{m}_{h}")
                            for h in range(NH)
                        ]
                        for i in range(IK):
                            st = ht_sb[:, i * TB + m * P: i * TB + (m + 1) * P]
                            for h in range(NH):
                                nc.tensor.matmul(
                                    y2[h][:],
                                    (st),
                                    (wd_sb[:, i * H + h * 512:
                                           i * H + (h + 1) * 512]),
                                    start=(i == 0),
                                    stop=(i == IK - 1),
                                )
                        for h in range(NH):
                            y_sb = evp.tile([P, 512], DT16, tag="ev",
                                            name=f"yevl{m}_{h}")
                            nc.vector.tensor_scalar_mul(
                                y_sb[:], y2[h][:], w_tiles[:, m:m + 1]
                            )
                            nc.sync.dma_start(
                                out=y_d[b * TB + m * P: b * TB + (m + 1) * P,
                                        h * 512:(h + 1) * 512],
                                in_=y_sb[:],
                            )

    nc.compile()
    return nc


def kernel(x, W_router, W_gate, W_up, W_down):
    global LAST_EXEC_NS
    if "nc" not in _CACHE:
        _CACHE["nc"] = _build()
    nc = _CACHE["nc"]

    bf16 = np.float16
    x_bf = np.asarray(x, dtype=np.float32).reshape(N, H).astype(bf16)
    xt = np.ascontiguousarray(x_bf.T)
    # repack router weights into the SBUF layout [128, HK*E]: row p holds
    # chunk k's rows (k*128+p) side by side -> plain contiguous DMA on device
    wr = np.ascontiguousarray(
        np.asarray(W_router, dtype=np.float32)
        .reshape(HK, P, E).transpose(1, 0, 2).reshape(P, HK * E)
    ).astype(bf16)
    eye = np.eye(E, dtype=np.float32)
    in_maps = []
    for e in range(E):
        # [ones | one-hot(e)] replicated across the 128 partitions
        sel = np.tile(
            np.concatenate([np.ones(E, dtype=np.float32), eye[e]]), (P, 1)
        )
        in_maps.append({
            "xt": xt,
            "wg": np.ascontiguousarray(W_gate[e]).astype(bf16),
            "wu": np.ascontiguousarray(W_up[e]).astype(bf16),
            "wd": np.ascontiguousarray(W_down[e]).astype(bf16),
            "wr": wr,
            "sel": np.ascontiguousarray(sel).astype(bf16),
        })

    res = run_bass_kernel_spmd(nc, in_maps, list(range(E)), trace=TRACE)
    LAST_EXEC_NS = res.exec_time_ns

    acc = np.zeros((N, H), dtype=np.float64)
    for r in res.results:
        acc += r["y"]
    return acc.astype(np.float32).reshape(x.shape[0], x.shape[1], H)


# revision 20
# speedup vs baseline: 1.0046x; 1.0035x over previous
"""MoE layer (dense all-experts SwiGLU + router-weighted sum) on 8 TRN2 cores.

Expert-parallel: core e holds expert e's weights (E=8). Every core sees the
full token stream x (shipped pre-transposed as xT [H, N]) and computes
  y_e = softmax(x @ W_router)[:, e] * ((silu(x@Wg_e) * (x@Wu_e)) @ Wd_e)
The host sums the 8 per-expert outputs.

All matmul operands are bf16 (inputs quantized on host, ~1e-3 rel err vs
the 2e-2 gate), PSUM accumulation fp32. bf16 halves SBUF so ALL weights
(Wg, Wu, Wd) are resident: no per-block weight streaming at all. The only
steady-state DMA is xt in (1 MB/block, sync HWDGE ring, 3 blocks deep) and
y out (2 MB/block, riding the otherwise-idle SWDGE ring; the last block
rides sync so the tail drains fast).

Per-core program, per 512-token block:
  router: logits^T [8,512] via PE (Wr stationary, xT moving), Exp on ACT,
          then per 128-token subtile a transpose-matmul with rhs=[ones|e_sel]
          gives [denom | numer] in PSUM -> w = numer * 1/denom on DVE.
  stage1: G/U [128i, 512tok] = Wg/Wu_chunk^T @ xT_chunk (8 K-chunks in PSUM),
          hT[i] = silu(G)*U -> SBUF bf16 (resident for the block, [I, tok]).
  stage2: Y[m] [128tok, 512h] accumulates over 16 i-chunks with hT as
          stationary and resident Wd slices as moving; evict = DVE multiply
          by the router weight, y DMA issued per tile.

Block 0 runs stage 1 k-OUTER in groups of 4 i-chunks (borrowing the psy
PSUM banks) so compute starts as soon as the first 512KB Wg k-chunk lands
(~5us) instead of waiting for the full Wg (~16us). Weight DMAs are issued
k-chunk-granular in consumption order: wr, xt(0), wg, xt(1), wu, wd, xt(2).

Scheduling notes:
  - tile-pool slot allocation order must match consumption order or the
    schedule deadlocks; all 8 xt chunks of 3 blocks are live at once.
  - next block's router runs between the two h-sweeps (PE filler).
"""
import numpy as np

import concourse.bass as bass
import concourse.mybir as mybir
import concourse.tile as tile
from concourse import bacc
from concourse.bass_utils import run_bass_kernel_spmd

P = 128
H, I, E = 1024, 2048, 8
N = 8192  # tokens = 4 * 2048
HK = H // P   # 8 contraction chunks over H
IK = I // P   # 16 chunks over I
TB = 512      # token block
NB = N // TB  # 16 blocks
NM = TB // P  # 4 token subtiles per block
NH = H // 512  # 2 output column halves
PF = 3        # xt prefetch depth (blocks)

F32 = mybir.dt.float32
DT16 = mybir.dt.float16
AF = mybir.ActivationFunctionType

# set by a driver (test.py) to profile; harness path keeps defaults
TRACE = False
LAST_EXEC_NS = None

_CACHE = {}


def _build():
    nc = bacc.Bacc("TRN2", target_bir_lowering=False, debug=False)

    xt_d = nc.dram_tensor("xt", [H, N], DT16, kind="ExternalInput").ap()
    wg_d = nc.dram_tensor("wg", [H, I], DT16, kind="ExternalInput").ap()
    wu_d = nc.dram_tensor("wu", [H, I], DT16, kind="ExternalInput").ap()
    wd_d = nc.dram_tensor("wd", [I, H], DT16, kind="ExternalInput").ap()
    wr_d = nc.dram_tensor("wr", [P, HK * E], DT16, kind="ExternalInput").ap()
    sel_d = nc.dram_tensor("sel", [P, 2 * E], DT16, kind="ExternalInput").ap()
    y_d = nc.dram_tensor("y", [N, H], DT16, kind="ExternalOutput").ap()

    with tile.TileContext(nc) as tc:
        with (
            tc.tile_pool(name="const", bufs=1) as const,
            tc.tile_pool(name="xtp", bufs=PF * HK) as xtp,
            tc.tile_pool(name="htp", bufs=1) as htp,
            tc.tile_pool(name="evp", bufs=8) as evp,
            tc.tile_pool(name="rtp", bufs=2) as rtp,
            tc.tile_pool(name="wp", bufs=2) as wp,
            tc.tile_pool(name="psgu", bufs=3, space="PSUM") as psgu,
            tc.tile_pool(name="psy", bufs=4, space="PSUM") as psy,
            tc.tile_pool(name="psr", bufs=1, space="PSUM") as psr,
        ):
            # resident weights: [128, HK*I] with chunk k at cols [k*I, (k+1)*I)
            wg_sb = const.tile([P, HK * I], DT16)
            wu_sb = const.tile([P, HK * I], DT16)
            # wd resident: [128, IK*H] with i-chunk at cols [i*H, (i+1)*H)
            wd_sb = const.tile([P, IK * H], DT16)
            wr_sb = const.tile([P, HK * E], DT16)
            sel_sb = const.tile([P, 2 * E], DT16)

            def load_xt(b, eng=None):
                eng = eng or nc.sync
                tok = slice(b * TB, (b + 1) * TB)
                chunks = []
                for k in range(HK):
                    ch = xtp.tile([P, TB], DT16, tag="xt", name=f"xt{b}_{k}")
                    eng.dma_start(
                        out=ch[:], in_=xt_d[k * P:(k + 1) * P, tok]
                    )
                    chunks.append(ch)
                return chunks

            # ---- prologue DMAs in consumption order. The HWDGE rings take
            # ~7us to wake at kernel start while SWDGE delivers in ~3us, so
            # the block-0-critical wr/xt0/sel ride SWDGE; the bulk weight
            # stream (wg halves, xt1, wu halves, wd) rides the sync ring.
            IH = I // 2
            nc.gpsimd.dma_start(out=wr_sb[:], in_=wr_d[:])
            xt_next = load_xt(0, eng=nc.gpsimd)
            nc.gpsimd.dma_start(out=sel_sb[:], in_=sel_d[:])
            # wg streamed as 256KB half-chunks, half-A (i 0-7) of every k
            # first: block-0's k-outer groups consume them as they land.
            for half in range(2):
                for k in range(HK):
                    nc.sync.dma_start(
                        out=wg_sb[:, k * I + half * IH: k * I + (half + 1) * IH],
                        in_=wg_d[k * P:(k + 1) * P, half * IH:(half + 1) * IH],
                    )
            xt_pre1 = load_xt(1)
            for half in range(2):
                for k in range(HK):
                    nc.sync.dma_start(
                        out=wu_sb[:, k * I + half * IH: k * I + (half + 1) * IH],
                        in_=wu_d[k * P:(k + 1) * P, half * IH:(half + 1) * IH],
                    )
            for j in range(IK // 2):  # wd as 512KB 2-chunk batches
                rows = slice(2 * j * P, (2 * j + 2) * P)
                nc.sync.dma_start(
                    out=wd_sb[:, 2 * j * H:(2 * j + 2) * H].rearrange(
                        "p (j c) -> p j c", j=2),
                    in_=wd_d[rows, :].rearrange("(j p) c -> p j c", p=P),
                )

            def router(xt_ch):
                # w[tok] = softmax(logits)[:, e] for one block. Only the 8
                # lt matmuls touch the PE: exp'd logits move to token-
                # partition layout via DVE 32x32 block transposes, and
                # numer/denom come from DVE free-dim accumulations against
                # selbc ([ones | one-hot(e)] replicated per partition).
                lt = psr.tile([E, TB], F32, tag="rt", name="lt")
                for k in range(HK):
                    nc.tensor.matmul(
                        lt[:],
                        (wr_sb[:, k * E:(k + 1) * E]),
                        (xt_ch[k][:]),
                        start=(k == 0),
                        stop=(k == HK - 1),
                    )
                # zero all 32 rows first (partition base must be 32-
                # aligned, and the transposes must not read uninitialized
                # SBUF), then exp overwrites rows 0-7.
                exp_sb = rtp.tile([32, TB], DT16, tag="exp", name="exp_sb")
                nc.vector.memset(exp_sb[:], 0.0)
                nc.scalar.activation(exp_sb[0:E, :], lt[:], AF.Exp)
                wv = wp.tile([P, NM], F32, tag="wv", name="wv")
                for m in range(NM):
                    et = rtp.tile([P, 32], DT16, tag="et", name="et", bufs=4)
                    for j in range(4):
                        nc.vector.transpose(
                            out=et[j * 32:(j + 1) * 32, 0:32],
                            in_=exp_sb[0:32, m * P + j * 32:
                                       m * P + (j + 1) * 32],
                        )
                    junk = wp.tile([P, E], F32, tag="junk", name="junk")
                    den = wp.tile([P, 1], F32, tag="den", name="den")
                    nc.vector.scalar_tensor_tensor(
                        out=junk[:], in0=et[:, 0:E], scalar=1.0,
                        in1=sel_sb[:, 0:E], op0=mybir.AluOpType.mult,
                        op1=mybir.AluOpType.mult, accum_out=den[:],
                    )
                    num = wp.tile([P, 1], F32, tag="num", name="num")
                    nc.vector.scalar_tensor_tensor(
                        out=junk[:], in0=et[:, 0:E], scalar=1.0,
                        in1=sel_sb[:, E:2 * E], op0=mybir.AluOpType.mult,
                        op1=mybir.AluOpType.mult, accum_out=num[:],
                    )
                    rec = wp.tile([P, 1], F32, tag="rec", name="rec")
                    nc.vector.reciprocal(rec[:], den[:])
                    nc.vector.tensor_tensor(
                        out=wv[:, m:m + 1], in0=num[:], in1=rec[:],
                        op=mybir.AluOpType.mult,
                    )
                return wv

            xtq = [xt_next, xt_pre1]  # blocks b, b+1 (already issued)
            w_next = router(xtq[0])
            for b in range(NB):
                xt_ch = xtq.pop(0)
                w_tiles = w_next
                if b + 2 < NB:
                    xtq.append(load_xt(b + 2))

                # ---- stage 1: hT[i] = silu(G)*U, [I-chunk, tok] layout
                ht_sb = htp.tile([P, IK * TB], DT16, tag="ht")

                def g_step(i):
                    g_ps = psgu.tile([P, TB], F32, tag="gu", name="g_ps")
                    for k in range(HK):
                        nc.tensor.matmul(
                            g_ps[:],
                            (wg_sb[:, k * I + i * P: k * I + (i + 1) * P]),
                            (xt_ch[k][:]),
                            start=(k == 0),
                            stop=(k == HK - 1),
                        )
                    nc.scalar.activation(
                        ht_sb[:, i * TB:(i + 1) * TB], g_ps[:], AF.Silu
                    )

                def u_step(i):
                    u_ps = psgu.tile([P, TB], F32, tag="gu", name="u_ps")
                    for k in range(HK):
                        nc.tensor.matmul(
                            u_ps[:],
                            (wu_sb[:, k * I + i * P: k * I + (i + 1) * P]),
                            (xt_ch[k][:]),
                            start=(k == 0),
                            stop=(k == HK - 1),
                        )
                    hsl = ht_sb[:, i * TB:(i + 1) * TB]
                    nc.vector.tensor_tensor(
                        out=hsl, in0=hsl, in1=u_ps[:], op=mybir.AluOpType.mult
                    )

                if b == 0:
                    # k-outer in groups of 4 i-chunks (borrowing psy banks):
                    # each 512KB wg k-chunk unlocks 4 matmuls as it lands.
                    for w_sb, is_g in ((wg_sb, True), (wu_sb, False)):
                        for grp in range(IK // 4):
                            ps4 = [
                                psy.tile([P, TB], F32, tag="y",
                                         name=f"b0_{'g' if is_g else 'u'}{grp}_{j}")
                                for j in range(4)
                            ]
                            for k in range(HK):
                                for j in range(4):
                                    i = grp * 4 + j
                                    nc.tensor.matmul(
                                        ps4[j][:],
                                        (w_sb[:, k * I + i * P:
                                              k * I + (i + 1) * P]),
                                        (xt_ch[k][:]),
                                        start=(k == 0),
                                        stop=(k == HK - 1),
                                    )
                            for j in range(4):
                                i = grp * 4 + j
                                hsl = ht_sb[:, i * TB:(i + 1) * TB]
                                if is_g:
                                    nc.scalar.activation(hsl, ps4[j][:], AF.Silu)
                                else:
                                    nc.vector.tensor_tensor(
                                        out=hsl, in0=hsl, in1=ps4[j][:],
                                        op=mybir.AluOpType.mult,
                                    )
                else:
                    for i in range(IK):
                        g_step(i)
                        u_step(i)

                # ---- stage 2: Y[m] [128tok, 512h] = hT^T @ Wd (resident),
                # scaled by w on evict; y DMA per tile on the idle SWDGE ring.
                if b + 1 < NB:
                    for h in range(NH):
                        if h == 1:
                            # next block's router between the h-sweeps
                            w_next = router(xtq[0])
                        y_ps = [
                            psy.tile([P, 512], F32, tag="y", name=f"y_ps{m}")
                            for m in range(NM)
                        ]
                        for i in range(IK):
                            rhs = wd_sb[:, i * H + h * 512:
                                        i * H + (h + 1) * 512]
                            for m in range(NM):
                                nc.tensor.matmul(
                                    y_ps[m][:],
                                    (ht_sb[:, i * TB + m * P:
                                           i * TB + (m + 1) * P]),
                                    (rhs),
                                    start=(i == 0),
                                    stop=(i == IK - 1),
                                )
                        for m in range(NM):
                            y_sb = evp.tile([P, 512], DT16, tag="ev",
                                            name=f"yev{h}_{m}")
                            nc.vector.tensor_scalar_mul(
                                y_sb[:], y_ps[m][:], w_tiles[:, m:m + 1]
                            )
                            nc.gpsimd.dma_start(
                                out=y_d[b * TB + m * P: b * TB + (m + 1) * P,
                                        h * 512:(h + 1) * 512],
                                in_=y_sb[:],
                            )
                else:
                    # last block: m-outer so earlier m's evict+DMA (sync
                    # ring, now idle) overlap later m's matmuls -> short tail
                    for m in range(NM):
                        y2 = [
                            psy.tile([P, 512], F32, tag="y", name=f"yl{m}_{h}")
                            for h in range(NH)
                        ]
                        for i in range(IK):
                            st = ht_sb[:, i * TB + m * P: i * TB + (m + 1) * P]
                            for h in range(NH):
                                nc.tensor.matmul(
                                    y2[h][:],
                                    (st),
                                    (wd_sb[:, i * H + h * 512:
                                           i * H + (h + 1) * 512]),
                                    start=(i == 0),
                                    stop=(i == IK - 1),
                                )
                        for h in range(NH):
                            y_sb = evp.tile([P, 512], DT16, tag="ev",
                                            name=f"yevl{m}_{h}")
                            nc.vector.tensor_scalar_mul(
                                y_sb[:], y2[h][:], w_tiles[:, m:m + 1]
                            )
                            nc.sync.dma_start(
                                out=y_d[b * TB + m * P: b * TB + (m + 1) * P,
                                        h * 512:(h + 1) * 512],
                                in_=y_sb[:],
                            )

    nc.compile()
    return nc


def kernel(x, W_router, W_gate, W_up, W_down):
    global LAST_EXEC_NS
    if "nc" not in _CACHE:
        _CACHE["nc"] = _build()
    nc = _CACHE["nc"]

    bf16 = np.float16
    x_bf = np.asarray(x, dtype=np.float32).reshape(N, H).astype(bf16)
    xt = np.ascontiguousarray(x_bf.T)
    # repack router weights into the SBUF layout [128, HK*E]: row p holds
    # chunk k's rows (k*128+p) side by side -> plain contiguous DMA on device
    wr = np.ascontiguousarray(
        np.asarray(W_router, dtype=np.float32)
        .reshape(HK, P, E).transpose(1, 0, 2).reshape(P, HK * E)
    ).astype(bf16)
    eye = np.eye(E, dtype=np.float32)
    in_maps = []
    for e in range(E):
        # [ones | one-hot(e)] replicated across the 128 partitions
        sel = np.tile(
            np.concatenate([np.ones(E, dtype=np.float32), eye[e]]), (P, 1)
        )
        in_maps.append({
            "xt": xt,
            "wg": np.ascontiguousarray(W_gate[e]).astype(bf16),
            "wu": np.ascontiguousarray(W_up[e]).astype(bf16),
            "wd": np.ascontiguousarray(W_down[e]).astype(bf16),
            "wr": wr,
            "sel": np.ascontiguousarray(sel).astype(bf16),
        })

    res = run_bass_kernel_spmd(nc, in_maps, list(range(E)), trace=TRACE)
    LAST_EXEC_NS = res.exec_time_ns

    acc = np.zeros((N, H), dtype=np.float64)
    for r in res.results:
        acc += r["y"]
    return acc.astype(np.float32).reshape(x.shape[0], x.shape[1], H)


# revision 23
# speedup vs baseline: 1.0053x; 1.0007x over previous
"""MoE layer (dense all-experts SwiGLU + router-weighted sum) on 8 TRN2 cores.

Expert-parallel: core e holds expert e's weights (E=8). Every core sees the
full token stream x (shipped pre-transposed as xT [H, N]) and computes
  y_e = softmax(x @ W_router)[:, e] * ((silu(x@Wg_e) * (x@Wu_e)) @ Wd_e)
The host sums the 8 per-expert outputs in float64.

All matmul operands are fp16: measured on TRN2, fp16 matmuls stream at the
full 2.4 GHz PE clock (216ns per 512-col matmul) while fp32r runs ~2.2 GHz
(233ns) and bf16 only ~2.0 GHz (259ns). fp16 quantization costs ~6e-4 rel
err vs the 2e-2 gate. PSUM accumulation stays fp32. 2-byte weights also fit
ALL of Wg/Wu/Wd resident in SBUF (96KB/partition), so there is no steady-
state weight streaming at all: per block only xt in (1MB, sync HWDGE ring,
3 blocks deep) and y out (1MB fp16 on the otherwise-idle SWDGE ring; the
last block rides sync and is m-outer so the tail drains in ~2us).

Per-core program, per 512-token block:
  router: logits^T [8,512] via PE (8 matmuls, the only PE work the router
          does), Exp on ACT, DVE 32x32 block-transposes to token-partition
          layout, then numer/denom as DVE free-dim reductions against a
          host-built [ones | one-hot(e)] selector; w = numer * 1/denom.
  stage1: G/U [128i, 512tok] = Wg/Wu_chunk^T @ xT_chunk (8 K-chunks into
          PSUM), hT[i] = silu(G)*U -> SBUF fp16 ([I, tok] layout).
  stage2: Y[m] [128tok, 512h] accumulates 16 i-chunks, hT stationary and
          resident Wd slices moving; evict = DVE multiply by w.

Block 0 runs stage 1 k-OUTER in groups of 4 i-chunks (borrowing psy PSUM
banks) so compute chases the 256KB Wg half-chunk DMAs as they land instead
of waiting for all of Wg. wr/xt0/sel ride SWDGE (first HWDGE completions
take ~7-10us at kernel start regardless of size).

Scheduling notes (hard-won):
  - tile-pool slot allocation order must match consumption order or the
    schedule deadlocks; 24 xt chunk tiles (3 blocks) are live at once.
  - next block's router runs between the two h-sweeps (PE filler).
  - psgu=3/psy=4/psr=1 PSUM split: 3 g/u banks decouple the g-start
    matmuls from silu completion latency.
  - never read uninitialized SBUF (the exp tile is memset before the
    transposes): it crashes the device, silently.
"""
import numpy as np

import concourse.bass as bass
import concourse.mybir as mybir
import concourse.tile as tile
from concourse import bacc
from concourse.bass_utils import run_bass_kernel_spmd

P = 128
H, I, E = 1024, 2048, 8
N = 8192  # tokens = 4 * 2048
HK = H // P   # 8 contraction chunks over H
IK = I // P   # 16 chunks over I
TB = 512      # token block
NB = N // TB  # 16 blocks
NM = TB // P  # 4 token subtiles per block
NH = H // 512  # 2 output column halves
PF = 3        # xt prefetch depth (blocks)

F32 = mybir.dt.float32
DT16 = mybir.dt.float16
AF = mybir.ActivationFunctionType

# set by a driver (test.py) to profile; harness path keeps defaults
TRACE = False
LAST_EXEC_NS = None

_CACHE = {}


def _build():
    nc = bacc.Bacc("TRN2", target_bir_lowering=False, debug=False)

    xt_d = nc.dram_tensor("xt", [H, N], DT16, kind="ExternalInput").ap()
    wg_d = nc.dram_tensor("wg", [H, I], DT16, kind="ExternalInput").ap()
    wu_d = nc.dram_tensor("wu", [H, I], DT16, kind="ExternalInput").ap()
    wd_d = nc.dram_tensor("wd", [I, H], DT16, kind="ExternalInput").ap()
    wr_d = nc.dram_tensor("wr", [P, HK * E], DT16, kind="ExternalInput").ap()
    sel_d = nc.dram_tensor("sel", [P, 3 * E], DT16, kind="ExternalInput").ap()
    y_d = nc.dram_tensor("y", [N, H], DT16, kind="ExternalOutput").ap()

    with tile.TileContext(nc) as tc:
        with (
            tc.tile_pool(name="const", bufs=1) as const,
            tc.tile_pool(name="xtp", bufs=PF * HK) as xtp,
            tc.tile_pool(name="htp", bufs=1) as htp,
            tc.tile_pool(name="evp", bufs=8) as evp,
            tc.tile_pool(name="rtp", bufs=2) as rtp,
            tc.tile_pool(name="wp", bufs=2) as wp,
            tc.tile_pool(name="psgu", bufs=3, space="PSUM") as psgu,
            tc.tile_pool(name="psy", bufs=5, space="PSUM") as psy,
        ):
            # resident weights: [128, HK*I] with chunk k at cols [k*I, (k+1)*I)
            wg_sb = const.tile([P, HK * I], DT16)
            wu_sb = const.tile([P, HK * I], DT16)
            # wd resident: [128, IK*H] with i-chunk at cols [i*H, (i+1)*H)
            wd_sb = const.tile([P, IK * H], DT16)
            wr_sb = const.tile([P, HK * E], DT16)
            sel_sb = const.tile([P, 3 * E], DT16)

            def load_xt(b, eng=None):
                eng = eng or nc.sync
                tok = slice(b * TB, (b + 1) * TB)
                chunks = []
                for k in range(HK):
                    ch = xtp.tile([P, TB], DT16, tag="xt", name=f"xt{b}_{k}")
                    eng.dma_start(
                        out=ch[:], in_=xt_d[k * P:(k + 1) * P, tok]
                    )
                    chunks.append(ch)
                return chunks

            # ---- prologue DMAs in consumption order. The HWDGE rings take
            # ~7us to wake at kernel start while SWDGE delivers in ~3us, so
            # the block-0-critical wr/xt0/sel ride SWDGE; the bulk weight
            # stream (wg halves, xt1, wu halves, wd) rides the sync ring.
            IH = I // 2
            nc.gpsimd.dma_start(out=wr_sb[:], in_=wr_d[:])
            xt_next = load_xt(0, eng=nc.gpsimd)
            nc.gpsimd.dma_start(out=sel_sb[:], in_=sel_d[:])
            # wg streamed as 256KB half-chunks, half-A (i 0-7) of every k
            # first: block-0's k-outer groups consume them as they land.
            for half in range(2):
                for k in range(HK):
                    nc.sync.dma_start(
                        out=wg_sb[:, k * I + half * IH: k * I + (half + 1) * IH],
                        in_=wg_d[k * P:(k + 1) * P, half * IH:(half + 1) * IH],
                    )
            xt_pre1 = load_xt(1)
            for half in range(2):
                for k in range(HK):
                    nc.sync.dma_start(
                        out=wu_sb[:, k * I + half * IH: k * I + (half + 1) * IH],
                        in_=wu_d[k * P:(k + 1) * P, half * IH:(half + 1) * IH],
                    )
            for j in range(IK // 2):  # wd as 512KB 2-chunk batches
                rows = slice(2 * j * P, (2 * j + 2) * P)
                nc.sync.dma_start(
                    out=wd_sb[:, 2 * j * H:(2 * j + 2) * H].rearrange(
                        "p (j c) -> p j c", j=2),
                    in_=wd_d[rows, :].rearrange("(j p) c -> p j c", p=P),
                )

            def router(xt_ch):
                # w[tok] = softmax(logits)[:, e] for one block. Only the 8
                # lt matmuls touch the PE: exp'd logits move to token-
                # partition layout via DVE 32x32 block transposes, and
                # numer/denom come from DVE free-dim accumulations against
                # selbc ([ones | one-hot(e)] replicated per partition).
                # 4-way column-tiled logits: col group j streams k-chunks
                # j and j+4 concurrently into partitions 32j..32j+7 of one
                # PSUM bank (~2 wave-times instead of 8 serial matmuls).
                # Only the first matmul clears the bank's has_written bits;
                # the other groups' first writes land on cleared bits and
                # overwrite, wave 2 accumulates.
                p4s = [
                    psy.tile([P, TB], F32, tag="y", name=f"lt4_{j}")
                    for j in range(4)
                ]
                for w in range(2):
                    for j in range(4):
                        k = 4 * w + j
                        nc.tensor.matmul(
                            p4s[j][32 * j:32 * j + E, :],
                            (wr_sb[:, k * E:(k + 1) * E]),
                            (xt_ch[k][:]),
                            start=(w == 0),
                            stop=(w == 1),
                            tile_position=(0, 32 * j),
                            skip_group_check=True,
                        )
                # collapse the 4 partition-group partials with one full-K
                # matmul against the group-sum selector. p4 is memset first:
                # the PE streams all 128 partitions and must never read
                # uninitialized SBUF (zero rows also zero out the garbage
                # via the selector's zero rows).
                p4 = rtp.tile([P, TB], DT16, tag="p4", name="p4")
                nc.vector.memset(p4[:], 0.0)
                for j in range(4):
                    nc.scalar.activation(
                        p4[32 * j:32 * j + E, :], p4s[j][32 * j:32 * j + E, :],
                        AF.Copy,
                    )
                lt = psgu.tile([E, TB], F32, tag="gu", name="lt")
                nc.tensor.matmul(
                    lt[:], (sel_sb[:, 2 * E:3 * E]), (p4[:]),
                    start=True, stop=True,
                )
                # zero all 32 rows first (partition base must be 32-
                # aligned, and the transposes must not read uninitialized
                # SBUF), then exp overwrites rows 0-7.
                exp_sb = rtp.tile([32, TB], DT16, tag="exp", name="exp_sb")
                nc.vector.memset(exp_sb[:], 0.0)
                nc.scalar.activation(exp_sb[0:E, :], lt[:], AF.Exp)
                wv = wp.tile([P, NM], F32, tag="wv", name="wv")
                for m in range(NM):
                    et = rtp.tile([P, 32], DT16, tag="et", name="et", bufs=4)
                    for j in range(4):
                        nc.vector.transpose(
                            out=et[j * 32:(j + 1) * 32, 0:32],
                            in_=exp_sb[0:32, m * P + j * 32:
                                       m * P + (j + 1) * 32],
                        )
                    junk = wp.tile([P, E], F32, tag="junk", name="junk")
                    den = wp.tile([P, 1], F32, tag="den", name="den")
                    nc.vector.scalar_tensor_tensor(
                        out=junk[:], in0=et[:, 0:E], scalar=1.0,
                        in1=sel_sb[:, 0:E], op0=mybir.AluOpType.mult,
                        op1=mybir.AluOpType.mult, accum_out=den[:],
                    )
                    num = wp.tile([P, 1], F32, tag="num", name="num")
                    nc.vector.scalar_tensor_tensor(
                        out=junk[:], in0=et[:, 0:E], scalar=1.0,
                        in1=sel_sb[:, E:2 * E], op0=mybir.AluOpType.mult,
                        op1=mybir.AluOpType.mult, accum_out=num[:],
                    )
                    rec = wp.tile([P, 1], F32, tag="rec", name="rec")
                    nc.vector.reciprocal(rec[:], den[:])
                    nc.vector.tensor_tensor(
                        out=wv[:, m:m + 1], in0=num[:], in1=rec[:],
                        op=mybir.AluOpType.mult,
                    )
                return wv

            xtq = [xt_next, xt_pre1]  # blocks b, b+1 (already issued)
            w_next = router(xtq[0])
            for b in range(NB):
                xt_ch = xtq.pop(0)
                w_tiles = w_next
                if b + 2 < NB:
                    xtq.append(load_xt(b + 2))

                # ---- stage 1: hT[i] = silu(G)*U, [I-chunk, tok] layout
                ht_sb = htp.tile([P, IK * TB], DT16, tag="ht")

                def g_step(i):
                    g_ps = psgu.tile([P, TB], F32, tag="gu", name="g_ps")
                    for k in range(HK):
                        nc.tensor.matmul(
                            g_ps[:],
                            (wg_sb[:, k * I + i * P: k * I + (i + 1) * P]),
                            (xt_ch[k][:]),
                            start=(k == 0),
                            stop=(k == HK - 1),
                        )
                    nc.scalar.activation(
                        ht_sb[:, i * TB:(i + 1) * TB], g_ps[:], AF.Silu
                    )

                def u_step(i):
                    u_ps = psgu.tile([P, TB], F32, tag="gu", name="u_ps")
                    for k in range(HK):
                        nc.tensor.matmul(
                            u_ps[:],
                            (wu_sb[:, k * I + i * P: k * I + (i + 1) * P]),
                            (xt_ch[k][:]),
                            start=(k == 0),
                            stop=(k == HK - 1),
                        )
                    hsl = ht_sb[:, i * TB:(i + 1) * TB]
                    nc.vector.tensor_tensor(
                        out=hsl, in0=hsl, in1=u_ps[:], op=mybir.AluOpType.mult
                    )

                if b == 0:
                    # k-outer in groups of 4 i-chunks (borrowing psy banks):
                    # each 512KB wg k-chunk unlocks 4 matmuls as it lands.
                    for w_sb, is_g in ((wg_sb, True), (wu_sb, False)):
                        for grp in range(IK // 4):
                            ps4 = [
                                psy.tile([P, TB], F32, tag="y",
                                         name=f"b0_{'g' if is_g else 'u'}{grp}_{j}")
                                for j in range(4)
                            ]
                            for k in range(HK):
                                for j in range(4):
                                    i = grp * 4 + j
                                    nc.tensor.matmul(
                                        ps4[j][:],
                                        (w_sb[:, k * I + i * P:
                                              k * I + (i + 1) * P]),
                                        (xt_ch[k][:]),
                                        start=(k == 0),
                                        stop=(k == HK - 1),
                                    )
                            for j in range(4):
                                i = grp * 4 + j
                                hsl = ht_sb[:, i * TB:(i + 1) * TB]
                                if is_g:
                                    nc.scalar.activation(hsl, ps4[j][:], AF.Silu)
                                else:
                                    nc.vector.tensor_tensor(
                                        out=hsl, in0=hsl, in1=ps4[j][:],
                                        op=mybir.AluOpType.mult,
                                    )
                else:
                    for i in range(IK):
                        g_step(i)
                        u_step(i)

                # ---- stage 2: Y[m] [128tok, 512h] = hT^T @ Wd (resident),
                # scaled by w on evict; y DMA per tile on the idle SWDGE ring.
                if b + 1 < NB:
                    w_next = router(xtq[0])
                    for h in range(NH):
                        y_ps = [
                            psy.tile([P, 512], F32, tag="y", name=f"y_ps{m}")
                            for m in range(NM)
                        ]
                        for i in range(IK):
                            rhs = wd_sb[:, i * H + h * 512:
                                        i * H + (h + 1) * 512]
                            for m in range(NM):
                                nc.tensor.matmul(
                                    y_ps[m][:],
                                    (ht_sb[:, i * TB + m * P:
                                           i * TB + (m + 1) * P]),
                                    (rhs),
                                    start=(i == 0),
                                    stop=(i == IK - 1),
                                )
                        for m in range(NM):
                            y_sb = evp.tile([P, 512], DT16, tag="ev",
                                            name=f"yev{h}_{m}")
                            nc.vector.tensor_scalar_mul(
                                y_sb[:], y_ps[m][:], w_tiles[:, m:m + 1]
                            )
                            nc.gpsimd.dma_start(
                                out=y_d[b * TB + m * P: b * TB + (m + 1) * P,
                                        h * 512:(h + 1) * 512],
                                in_=y_sb[:],
                            )
                else:
                    # last block: m-outer so earlier m's evict+DMA (sync
                    # ring, now idle) overlap later m's matmuls -> short tail
                    for m in range(NM):
                        y2 = [
                            psy.tile([P, 512], F32, tag="y", name=f"yl{m}_{h}")
                            for h in range(NH)
                        ]
                        for i in range(IK):
                            st = ht_sb[:, i * TB + m * P: i * TB + (m + 1) * P]
                            for h in range(NH):
                                nc.tensor.matmul(
                                    y2[h][:],
                                    (st),
                                    (wd_sb[:, i * H + h * 512:
                                           i * H + (h + 1) * 512]),
                                    start=(i == 0),
                                    stop=(i == IK - 1),
                                )
                        for h in range(NH):
                            y_sb = evp.tile([P, 512], DT16, tag="ev",
                                            name=f"yevl{m}_{h}")
                            nc.vector.tensor_scalar_mul(
                                y_sb[:], y2[h][:], w_tiles[:, m:m + 1]
                            )
                            nc.sync.dma_start(
                                out=y_d[b * TB + m * P: b * TB + (m + 1) * P,
                                        h * 512:(h + 1) * 512],
                                in_=y_sb[:],
                            )

    nc.compile()
    return nc


def kernel(x, W_router, W_gate, W_up, W_down):
    global LAST_EXEC_NS
    if "nc" not in _CACHE:
        _CACHE["nc"] = _build()
    nc = _CACHE["nc"]

    bf16 = np.float16
    x_bf = np.asarray(x, dtype=np.float32).reshape(N, H).astype(bf16)
    xt = np.ascontiguousarray(x_bf.T)
    # repack router weights into the SBUF layout [128, HK*E]: row p holds
    # chunk k's rows (k*128+p) side by side -> plain contiguous DMA on device
    wr = np.ascontiguousarray(
        np.asarray(W_router, dtype=np.float32)
        .reshape(HK, P, E).transpose(1, 0, 2).reshape(P, HK * E)
    ).astype(bf16)
    eye = np.eye(E, dtype=np.float32)
    in_maps = []
    for e in range(E):
        # [ones | one-hot(e)] replicated across the 128 partitions, plus
        # the column-group sum selector s4[p, :] = one-hot(p % 32) for
        # p % 32 < 8 (zero rows elsewhere)
        s4 = ((np.arange(P) % 32)[:, None] == np.arange(E)[None, :])
        sel = np.concatenate([
            np.tile(np.concatenate([np.ones(E, np.float32), eye[e]]), (P, 1)),
            s4.astype(np.float32),
        ], axis=1)
        in_maps.append({
            "xt": xt,
            "wg": np.ascontiguousarray(W_gate[e]).astype(bf16),
            "wu": np.ascontiguousarray(W_up[e]).astype(bf16),
            "wd": np.ascontiguousarray(W_down[e]).astype(bf16),
            "wr": wr,
            "sel": np.ascontiguousarray(sel).astype(bf16),
        })

    res = run_bass_kernel_spmd(nc, in_maps, list(range(E)), trace=TRACE)
    LAST_EXEC_NS = res.exec_time_ns

    acc = np.zeros((N, H), dtype=np.float64)
    for r in res.results:
        acc += r["y"]
    return acc.astype(np.float32).reshape(x.shape[0], x.shape[1], H)
